# revision 7
# baseline (speedup 1.0000x reference)
"""Trainium2 Bass kernel for causal cosine-sim attention block (8 cores), v6.

Reference computation:
  x [2, 2048, 1024] fp32
  xn = LayerNorm(x) * ln_w + ln_b
  qkv = xn @ W_qkv -> q, k, v   (16 heads x 64)
  q, k l2-normalized per head-dim; sim = (q.k) * 8, causal mask, softmax
  o = attn @ v ; out = o @ W_out   [2, 2048, 1024] fp32

Sharding (8 cores):
  - QKV projection + attention: head-parallel. Core c owns heads {2c, 2c+1}
    (column-sharded W_qkv).
  - Out projection: token-parallel over strided 128-token granules:
    granule g (tokens [128g, 128g+128)) of each batch-half belongs to core
    g % 8. Four small AllToAlls (one per batch-half) exchange head-sharded
    o^T for token granules, each overlapping subsequent attention compute.

v6 structure notes:
  - LN mean-subtraction is folded into W on the host (column-centered W);
    no on-chip rank-1 correction at all. A 385th W column of 1/1024 makes
    the QKV psum's last column the per-token mean (needed for variance).
  - rstd (for the V scale; it cancels for q,k under l2-norm) comes from
    sum(x^2) computed on DVE from x rows; var = ssq/1024 - mu^2.
  - x is fed twice from HBM: token-row tiles (for x^2) and transposed
    stripes x^T (matmul lhsT) -- no on-chip x transposes.
  - q is scaled by 1/||q|| on DVE; k stays raw and 8/||k|| rides the Exp
    activation's per-partition scale operand.
  - Softmax denominators (from the [V|1] PV matmul) are normalized via
    reciprocal + K=1 ones-matmul broadcast + DVE multiply.
  - PE warm-up: dummy matmuls at kernel start trip the HAM clock gate
    (cold 1.2 GHz -> warm 2.4 GHz) while input DMAs run.
  - Schedule: QKV(b0) -> [attention(b0) || QKV(b1)] -> [attention(b1) ||
    A2A(b0)+outproj(b0)] -> per-half A2A(b1)/outproj(b1) pipeline.
"""

import numpy as np

import concourse.bass as bass
import concourse.mybir as mybir
import concourse.tile as tile
from concourse import bacc
from concourse.bass import ts, ds

F32 = mybir.dt.float32
BF16 = mybir.dt.bfloat16

NCORES = 8
DIM = 1024
HEADS = 16
DHEAD = 64
INNER = HEADS * DHEAD          # 1024
B = 2
N = 2048
NTOK = B * N                   # 4096
HLOC = HEADS // NCORES         # 2 heads per core
QKV_COLS = 3 * HLOC * DHEAD    # 384
EPS = 1e-5
SCALE = 8.0
P = 128
KT_PER_B = N // P              # 16 token tiles per batch
QB_PER_B = N // 512            # 4 q-blocks (512) per batch
GRAN = 128                     # out-proj token granule
AluOp = mybir.AluOpType
Act = mybir.ActivationFunctionType
AxX = mybir.AxisListType.X


def build_kernel():
    nc = bacc.Bacc("TRN2", target_bir_lowering=False, debug=False,
                   num_devices=NCORES)

    x_rows = nc.dram_tensor("x_rows", [NTOK, DIM], BF16, kind="ExternalInput")
    x_tr = nc.dram_tensor("x_tr", [P, DIM // P, NTOK], BF16,
                          kind="ExternalInput")
    w_qkv = nc.dram_tensor("w_qkv", [P, DIM // P, QKV_COLS + 1], BF16,
                           kind="ExternalInput")
    w_out = nc.dram_tensor("w_out", [P, INNER // P, DIM], BF16,
                           kind="ExternalInput")
    y_out = nc.dram_tensor("y_out", [B, 2, GRAN, DIM], BF16,
                           kind="ExternalOutput")

    with tile.TileContext(nc) as tc:
        _body(nc, tc, x_rows, x_tr, w_qkv, w_out, y_out)
    nc.compile()
    return nc


def _body(nc, tc, x_rows, x_tr, w_qkv, w_out, y_out):
    import contextlib
    ctx = contextlib.ExitStack()
    with ctx:
        persist = ctx.enter_context(tc.tile_pool(name="persist", bufs=1))
        xt_pool = ctx.enter_context(tc.tile_pool(name="xt", bufs=2))
        xr_pool = ctx.enter_context(tc.tile_pool(name="xr", bufs=2))
        sq_pool = ctx.enter_context(tc.tile_pool(name="sqd", bufs=2))
        qk_pool = ctx.enter_context(tc.tile_pool(name="qks", bufs=3))
        small = ctx.enter_context(tc.tile_pool(name="small", bufs=4))
        mm_ps = ctx.enter_context(
            tc.tile_pool(name="mmps", bufs=2, space="PSUM"))
        st_ps_pool = ctx.enter_context(
            tc.tile_pool(name="stps", bufs=2, space="PSUM"))
        o_ps_pool = ctx.enter_context(
            tc.tile_pool(name="ops", bufs=1, space="PSUM"))
        bc_ps_pool = ctx.enter_context(
            tc.tile_pool(name="bcps", bufs=1, space="PSUM"))
        warm_ps_pool = ctx.enter_context(
            tc.tile_pool(name="warmps", bufs=1, space="PSUM"))
        e_pool = ctx.enter_context(tc.tile_pool(name="epool", bufs=4))
        oU_pool = ctx.enter_context(tc.tile_pool(name="oU", bufs=2))
        oT_pool = ctx.enter_context(tc.tile_pool(name="oT", bufs=2))
        oA_pool = ctx.enter_context(tc.tile_pool(name="oA", bufs=2))
        out_pool = ctx.enter_context(tc.tile_pool(name="outp", bufs=2))
        norm_pool = ctx.enter_context(tc.tile_pool(name="norm", bufs=4))
        dram = ctx.enter_context(tc.tile_pool(name="dram", bufs=1,
                                              space="DRAM"))

        # ---------------- persistent SBUF ----------------
        w_qkv_sb = persist.tile([P, DIM // P, QKV_COLS + 1], BF16)
        w_out_sb = persist.tile([P, INNER // P, DIM], BF16)
        qkT = persist.tile([P, 2, B, N], BF16)                    # 16 KB
        v_sb = persist.tile([P, B, KT_PER_B, HLOC, DHEAD + 1], BF16)
        rk8_all = persist.tile([P, B, KT_PER_B, HLOC], F32)
        eps_t = persist.tile([P, 1], F32)
        ones1 = persist.tile([1, DHEAD], BF16)
        warm_a = persist.tile([P, P], BF16)
        warm_b = persist.tile([P, 512], BF16)

        # upfront DMAs (Sync queue) + constants
        nc.sync.dma_start(w_qkv_sb[:], w_qkv.ap())
        nc.sync.dma_start(w_out_sb[:], w_out.ap())
        nc.vector.memset(eps_t[:], EPS)
        nc.vector.memset(ones1[:], 1.0)
        nc.vector.memset(warm_a[:], 0.0)
        nc.vector.memset(warm_b[:], 0.0)
        nc.vector.memset(v_sb[:, :, :, :, DHEAD], 1.0)

        # whole-batch x^T stripes (fat DMAs, both issued immediately)
        xt_sb = []
        for bi in range(B):
            t = xt_pool.tile([P, DIM // P, N], BF16, tag="xt")
            nc.sync.dma_start(t[:], x_tr.ap()[:, :, ds(bi * N, N)])
            xt_sb.append(t)

        # x token-row groups (512 tokens each) on the Scalar queue
        xr_view = x_rows.ap().rearrange("(n p) d -> p n d", p=P)

        # PE warm-up: dummy matmuls while DMAs land (~6 us of PE busy)
        warm_ps = warm_ps_pool.tile([P, 512], F32, tag="warm")
        for i in range(14):
            nc.tensor.matmul(warm_ps[:], lhsT=warm_a[:], rhs=warm_b[:],
                             start=True, stop=True)

        # DRAM staging for the 4 AllToAlls (one per batch-half)
        cc_in = [[None] * 2 for _ in range(B)]
        cc_out = [[None] * 2 for _ in range(B)]
        for bi in range(B):
            for h in range(2):
                cc_in[bi][h] = dram.tile([NCORES, P, GRAN], BF16,
                                         name=f"cci{bi}{h}")
                cc_out[bi][h] = dram.tile([NCORES, P, GRAN], BF16,
                                          name=f"cco{bi}{h}")

        oT_b = [None] * B      # per-batch o^T (normalized, bf16)

        # ---------------- stage A: QKV for one 128-token tile ------------
        def qkv_tile(bi, ti):
            i = bi * KT_PER_B + ti
            g, j = divmod(i, 4)     # 512-token group / tile within group
            if j == 0:
                qkv_tile.xr = xr_pool.tile([P, 4, DIM], BF16, tag="xr")
                nc.scalar.dma_start(qkv_tile.xr[:],
                                    xr_view[:, ds(4 * g, 4), :])
            xr = qkv_tile.xr

            qkv_ps = mm_ps.tile([P, QKV_COLS + 1], F32, tag="mm")
            for o in range(DIM // P):
                nc.tensor.matmul(qkv_ps[:],
                                 lhsT=xt_sb[bi][:, o, ts(ti, P)],
                                 rhs=w_qkv_sb[:, o, :],
                                 start=(o == 0), stop=(o == DIM // P - 1))

            # sum(x^2) for rstd (DVE square + reduce on the x rows)
            sqd = sq_pool.tile([P, DIM], BF16, tag="sqd")
            nc.vector.tensor_tensor(sqd[:], xr[:, j, :], xr[:, j, :],
                                    AluOp.mult)
            ssqx = small.tile([P, 1], F32, tag="ssqx")
            nc.vector.reduce_sum(ssqx[:], sqd[:], axis=AxX)

            # rstd = 1/sqrt(ssq/1024 - mu^2 + eps); mu = psum col 384
            mu = small.tile([P, 1], F32, tag="mu")
            nc.vector.tensor_copy(mu[:], qkv_ps[:, QKV_COLS:QKV_COLS + 1])
            musq = small.tile([P, 1], F32, tag="musq")
            nc.vector.tensor_tensor(musq[:], mu[:], mu[:], AluOp.mult)
            bias_t = small.tile([P, 1], F32, tag="biast")
            nc.vector.tensor_scalar(bias_t[:], musq[:], -1.0, EPS,
                                    AluOp.mult, AluOp.add)
            rstd = small.tile([P, 1], F32, tag="rstd")
            nc.scalar.activation(rstd[:], ssqx[:], Act.Sqrt,
                                 bias=bias_t[:], scale=1.0 / DIM)
            nc.vector.reciprocal(rstd[:], rstd[:])

            # v = rstd * v_cols (psum -> bf16, per-partition scale)
            nc.vector.tensor_scalar_mul(
                v_sb[:, bi, ti, :, 0:DHEAD],
                qkv_ps[:, 2 * P:2 * P + 2 * DHEAD]
                .rearrange("p (h d) -> p h d", d=DHEAD),
                rstd[:])

            # q|k -> bf16 scratch; squared norms per 64-col group
            qk = qk_pool.tile([P, 2 * P], BF16, tag="qk")
            nc.vector.tensor_copy(qk[:], qkv_ps[:, 0:2 * P])
            sq = qk_pool.tile([P, 2 * P], BF16, tag="sq")
            nc.vector.tensor_tensor(sq[:], qk[:], qk[:], AluOp.mult)
            ssq = small.tile([P, 4], F32, tag="ssq")
            nc.vector.reduce_sum(
                ssq[:], sq[:].rearrange("p (j d) -> p j d", d=DHEAD),
                axis=AxX)

            # rq = 1/max(||q||, 1e-12); rk8 = 8/max(||k||, 1e-12)
            rq = small.tile([P, 2], F32, tag="rq")
            nc.scalar.activation(rq[:], ssq[:, 0:2], Act.Sqrt)
            nc.vector.tensor_scalar_max(rq[:], rq[:], 1e-12)
            nc.vector.reciprocal(rq[:], rq[:])
            rk = small.tile([P, 2], F32, tag="rk")
            nc.scalar.activation(rk[:], ssq[:, 2:4], Act.Sqrt,
                                 scale=1.0 / (SCALE * SCALE))
            nc.vector.tensor_scalar_max(rk[:], rk[:], 1e-12 / SCALE)
            nc.vector.reciprocal(rk8_all[:, bi, ti, :], rk[:])

            # q-hat in place, then transpose q|k to qkT (Sync queue)
            for hh in range(HLOC):
                nc.vector.tensor_scalar_mul(
                    qk[:, ts(hh, DHEAD)], qk[:, ts(hh, DHEAD)],
                    rq[:, hh:hh + 1])
            nc.sync.dma_start_transpose(qkT[:, :, bi, ts(ti, P)], qk[:])

        # ---------------- stage B: attention for one 512-q block ---------
        def attn_qblock(bi, qb, oU):
            o_ps = []
            for hh in range(HLOC):
                o_ps.append(o_ps_pool.tile([1 + DHEAD, 512], F32,
                                           tag=f"ops{hh}", name=f"ops{hh}"))
            nkt = 4 * (qb + 1)
            for kt in range(nkt):
                d = kt - 4 * qb
                c0 = max(d, 0) * P
                for hh in range(HLOC):
                    hsl = slice(hh * DHEAD, (hh + 1) * DHEAD)
                    st_ps = st_ps_pool.tile([P, 512], F32, tag="stps")
                    nc.tensor.matmul(
                        st_ps[:], lhsT=qkT[hsl, 1, bi, ts(kt, P)],
                        rhs=qkT[hsl, 0, bi, ds(qb * 512, 512)],
                        start=True, stop=True,
                        tile_position=(hh * DHEAD, 0))
                    e_t = e_pool.tile([P, 512], BF16, tag="et")
                    nc.scalar.activation(e_t[:, c0:512], st_ps[:, c0:512],
                                         Act.Exp,
                                         scale=rk8_all[:, bi, kt,
                                                       hh:hh + 1])
                    if d >= 0:
                        nc.gpsimd.affine_select(
                            out=e_t[:, c0:c0 + P], in_=e_t[:, c0:c0 + P],
                            pattern=[[1, P]], compare_op=AluOp.is_ge,
                            fill=0.0, base=0, channel_multiplier=-1)
                    nc.tensor.matmul(
                        o_ps[hh][:, c0:512],
                        lhsT=v_sb[:, bi, kt, hh, :],
                        rhs=e_t[:, c0:512],
                        start=(kt == 0), stop=(kt == nkt - 1))
            for hh in range(HLOC):
                nc.vector.tensor_copy(oU[:, qb % 2, hh, :], o_ps[hh][:])

        # ------- stage C: normalize half-batch, A2A, (out-proj later) ----
        def norm_half(bi, h, oU):
            for q2 in range(2):
                qb = 2 * h + q2
                for hh in range(HLOC):
                    rden = norm_pool.tile([1, 512], BF16, tag="rden")
                    with nc.allow_low_precision(
                            reason="bf16 softmax denom, rel-err budget"):
                        nc.vector.reciprocal(
                            rden[:], oU[DHEAD:DHEAD + 1, q2, hh, :])
                    bc_ps = bc_ps_pool.tile([DHEAD, 512], F32, tag="bc")
                    nc.tensor.matmul(bc_ps[:], lhsT=ones1[:], rhs=rden[:],
                                     start=True, stop=True)
                    nc.vector.tensor_tensor(
                        oT_b[bi][hh * DHEAD:(hh + 1) * DHEAD,
                                 ds(qb * 512, 512)],
                        oU[0:DHEAD, q2, hh, :], bc_ps[:], AluOp.mult)
            nc.sync.dma_start(
                cc_in[bi][h][:].rearrange("s p f -> p s f"),
                oT_b[bi][:, ds(h * 1024, 1024)]
                .rearrange("p (s f) -> p s f", f=GRAN))
            nc.gpsimd.collective_compute(
                "AllToAll", AluOp.bypass,
                replica_groups=[list(range(NCORES))],
                ins=[cc_in[bi][h].opt()], outs=[cc_out[bi][h].opt()])

        def outproj_half(bi, h):
            oA = oA_pool.tile([P, INNER // P, GRAN], BF16, tag="oA")
            nc.sync.dma_start(oA[:],
                              cc_out[bi][h][:].rearrange("s p f -> p s f"))
            yt = out_pool.tile([P, DIM], BF16, tag="yt")
            for half in range(2):
                out_ps = mm_ps.tile([P, 512], F32, tag="mm")
                for o in range(INNER // P):
                    nc.tensor.matmul(
                        out_ps[:], lhsT=oA[:, o, :],
                        rhs=w_out_sb[:, o, ds(half * 512, 512)],
                        start=(o == 0), stop=(o == INNER // P - 1))
                nc.vector.tensor_copy(yt[:, ds(half * 512, 512)], out_ps[:])
            nc.sync.dma_start(y_out.ap()[bi, h], yt[:])

        # ---------------- the schedule ----------------
        for ti in range(KT_PER_B):              # QKV batch 0
            qkv_tile(0, ti)

        oU0 = oU_pool.tile([1 + DHEAD, 2, HLOC, 512], F32, tag="oU")
        oT_b[0] = oT_pool.tile([P, N], BF16, tag="oTb", name="oT0")
        for qb in range(QB_PER_B):              # attn(b0) || QKV(b1)
            if qb == 2:
                oU0b = oU_pool.tile([1 + DHEAD, 2, HLOC, 512], F32,
                                    tag="oU")
            attn_qblock(0, qb, oU0 if qb < 2 else oU0b)
            for ti in range(4 * qb, 4 * qb + 4):
                qkv_tile(1, ti)
            if qb == 1:
                norm_half(0, 0, oU0)
        norm_half(0, 1, oU0b)

        oU1 = oU_pool.tile([1 + DHEAD, 2, HLOC, 512], F32, tag="oU")
        oT_b[1] = oT_pool.tile([P, N], BF16, tag="oTb", name="oT1")
        attn_qblock(1, 0, oU1)
        attn_qblock(1, 1, oU1)
        outproj_half(0, 0)
        norm_half(1, 0, oU1)
        oU1b = oU_pool.tile([1 + DHEAD, 2, HLOC, 512], F32, tag="oU")
        attn_qblock(1, 2, oU1b)
        outproj_half(0, 1)
        attn_qblock(1, 3, oU1b)
        norm_half(1, 1, oU1b)
        outproj_half(1, 0)
        outproj_half(1, 1)


# ----------------------------------------------------------------------
# Host side
# ----------------------------------------------------------------------

def make_in_maps(x, ln_w, ln_b, W_qkv, W_out):
    """Build the per-core input maps (host-side sharding/marshaling)."""
    import ml_dtypes
    x = np.asarray(x, dtype=np.float32)
    ln_w = np.asarray(ln_w, dtype=np.float32)
    ln_b = np.asarray(ln_b, dtype=np.float32)
    W_qkv = np.asarray(W_qkv, dtype=np.float32)
    W_out = np.asarray(W_out, dtype=np.float32)

    assert np.allclose(ln_b, 0.0), \
        "kernel folds ln_b@W into a bias; nonzero ln_b not wired up"

    x2d = np.ascontiguousarray(x.reshape(NTOK, DIM))
    x_rows = x2d.astype(ml_dtypes.bfloat16)
    # x^T stripes: [128 p, 8 o, 4096 t] with d = 128*o + p
    x_tr = np.ascontiguousarray(
        x2d.T.reshape(DIM // P, P, NTOK).transpose(1, 0, 2)
    ).astype(ml_dtypes.bfloat16)

    w_eff = ln_w[:, None] * W_qkv  # [1024, 3072]
    q_w = w_eff[:, 0 * INNER:1 * INNER]
    k_w = w_eff[:, 1 * INNER:2 * INNER]
    v_w = w_eff[:, 2 * INNER:3 * INNER]
    w_out_r = np.ascontiguousarray(
        W_out.reshape(INNER // P, P, DIM).transpose(1, 0, 2)
    ).astype(ml_dtypes.bfloat16)

    in_maps = []
    for c in range(NCORES):
        h0, h1 = 2 * c, 2 * c + 2
        wq = q_w[:, h0 * DHEAD:h1 * DHEAD]
        wk = k_w[:, h0 * DHEAD:h1 * DHEAD]
        wv = v_w[:, h0 * DHEAD:h1 * DHEAD]
        w_c = np.concatenate([wq, wk, wv], axis=1)      # [1024, 384]
        w_c = w_c - w_c.mean(axis=0, keepdims=True)     # fold LN mean-sub
        mu_col = np.full((DIM, 1), 1.0 / DIM, dtype=np.float32)
        w_c = np.concatenate([w_c, mu_col], axis=1)     # [1024, 385]
        w_c = np.ascontiguousarray(
            w_c.reshape(DIM // P, P, QKV_COLS + 1).transpose(1, 0, 2)
        ).astype(ml_dtypes.bfloat16)
        in_maps.append({
            "x_rows": x_rows,
            "x_tr": x_tr,
            "w_qkv": w_c,
            "w_out": w_out_r,
        })
    return in_maps


def gather_output(results):
    """results: list of per-core {name: array} -> full [2, 2048, 1024]."""
    full = np.empty((B, N, DIM), dtype=np.float32)
    for c in range(NCORES):
        part = np.asarray(results[c]["y_out"], dtype=np.float32)
        for bi in range(B):
            for h in range(2):
                t0 = h * 1024 + c * GRAN
                full[bi, t0:t0 + GRAN, :] = part[bi, h]
    return full


_NC_CACHE = None


def kernel(x, ln_w, ln_b, W_qkv, W_out):
    global _NC_CACHE
    from concourse.bass_utils import run_bass_kernel_spmd
    if _NC_CACHE is None:
        _NC_CACHE = build_kernel()
    in_maps = make_in_maps(x, ln_w, ln_b, W_qkv, W_out)
    res = run_bass_kernel_spmd(_NC_CACHE, in_maps,
                               core_ids=list(range(NCORES)))
    return gather_output(res.results)


# revision 13
# speedup vs baseline: 1.1717x; 1.1717x over previous
"""Trainium2 Bass kernel for causal cosine-sim attention block (8 cores), v6.

Reference computation:
  x [2, 2048, 1024] fp32
  xn = LayerNorm(x) * ln_w + ln_b
  qkv = xn @ W_qkv -> q, k, v   (16 heads x 64)
  q, k l2-normalized per head-dim; sim = (q.k) * 8, causal mask, softmax
  o = attn @ v ; out = o @ W_out   [2, 2048, 1024] fp32

Sharding (8 cores):
  - QKV projection + attention: head-parallel. Core c owns heads {2c, 2c+1}
    (column-sharded W_qkv).
  - Out projection: token-parallel over strided 128-token granules:
    granule g (tokens [128g, 128g+128)) of each batch-half belongs to core
    g % 8. Four small AllToAlls (one per batch-half) exchange head-sharded
    o^T for token granules, each overlapping subsequent attention compute.

v6 structure notes:
  - LN mean-subtraction is folded into W on the host (column-centered W);
    no on-chip rank-1 correction at all. A 385th W column of 1/1024 makes
    the QKV psum's last column the per-token mean (needed for variance).
  - rstd (for the V scale; it cancels for q,k under l2-norm) comes from
    sum(x^2) computed on DVE from x rows; var = ssq/1024 - mu^2.
  - x is fed twice from HBM: token-row tiles (for x^2) and transposed
    stripes x^T (matmul lhsT) -- no on-chip x transposes.
  - q is scaled by 1/||q|| on DVE; k stays raw and 8/||k|| rides the Exp
    activation's per-partition scale operand.
  - Softmax denominators (from the [V|1] PV matmul) are normalized via
    reciprocal + K=1 ones-matmul broadcast + DVE multiply.
  - PE warm-up: dummy matmuls at kernel start trip the HAM clock gate
    (cold 1.2 GHz -> warm 2.4 GHz) while input DMAs run.
  - Schedule: QKV(b0) -> [attention(b0) || QKV(b1)] -> [attention(b1) ||
    A2A(b0)+outproj(b0)] -> per-half A2A(b1)/outproj(b1) pipeline.
"""

import numpy as np

import concourse.bass as bass
import concourse.mybir as mybir
import concourse.tile as tile
from concourse import bacc
from concourse.bass import ts, ds

F32 = mybir.dt.float32
BF16 = mybir.dt.bfloat16

NCORES = 8
DIM = 1024
HEADS = 16
DHEAD = 64
INNER = HEADS * DHEAD          # 1024
B = 2
N = 2048
NTOK = B * N                   # 4096
HLOC = HEADS // NCORES         # 2 heads per core
QKV_COLS = 3 * HLOC * DHEAD    # 384
EPS = 1e-5
SCALE = 8.0
P = 128
KT_PER_B = N // P              # 16 token tiles per batch
QB_PER_B = N // 512            # 4 q-blocks (512) per batch
GRAN = 128                     # out-proj token granule
AluOp = mybir.AluOpType
Act = mybir.ActivationFunctionType
AxX = mybir.AxisListType.X


def build_kernel():
    nc = bacc.Bacc("TRN2", target_bir_lowering=False, debug=False,
                   num_devices=NCORES)

    x_rows = nc.dram_tensor("x_rows", [NTOK, DIM], BF16, kind="ExternalInput")
    x_tr = nc.dram_tensor("x_tr", [P, DIM // P, NTOK], BF16,
                          kind="ExternalInput")
    w_qkv = nc.dram_tensor("w_qkv", [P, DIM // P, QKV_COLS + 1], BF16,
                           kind="ExternalInput")
    w_out = nc.dram_tensor("w_out", [P, INNER // P, DIM], BF16,
                           kind="ExternalInput")
    y_out = nc.dram_tensor("y_out", [B, 2, GRAN, DIM], BF16,
                           kind="ExternalOutput")

    with tile.TileContext(nc) as tc:
        _body(nc, tc, x_rows, x_tr, w_qkv, w_out, y_out)
    nc.compile()
    return nc


def _body(nc, tc, x_rows, x_tr, w_qkv, w_out, y_out):
    import contextlib
    ctx = contextlib.ExitStack()
    with ctx:
        persist = ctx.enter_context(tc.tile_pool(name="persist", bufs=1))
        xt_pool = ctx.enter_context(tc.tile_pool(name="xt", bufs=2))
        xr_pool = ctx.enter_context(tc.tile_pool(name="xr", bufs=2))
        sq_pool = ctx.enter_context(tc.tile_pool(name="sqd", bufs=2))
        qk_pool = ctx.enter_context(tc.tile_pool(name="qks", bufs=2))
        small = ctx.enter_context(tc.tile_pool(name="small", bufs=4))
        mm_ps = ctx.enter_context(
            tc.tile_pool(name="mmps", bufs=2, space="PSUM"))
        st_ps_pool = ctx.enter_context(
            tc.tile_pool(name="stps", bufs=2, space="PSUM"))
        o_ps_pool = ctx.enter_context(
            tc.tile_pool(name="ops", bufs=1, space="PSUM"))
        bc_ps_pool = ctx.enter_context(
            tc.tile_pool(name="bcps", bufs=1, space="PSUM"))
        warm_ps_pool = ctx.enter_context(
            tc.tile_pool(name="warmps", bufs=1, space="PSUM"))
        e_pool = ctx.enter_context(tc.tile_pool(name="epool", bufs=4))
        oU_pool = ctx.enter_context(tc.tile_pool(name="oU", bufs=2))
        oT_pool = ctx.enter_context(tc.tile_pool(name="oT", bufs=2))
        oA_pool = ctx.enter_context(tc.tile_pool(name="oA", bufs=2))
        out_pool = ctx.enter_context(tc.tile_pool(name="outp", bufs=2))
        norm_pool = ctx.enter_context(tc.tile_pool(name="norm", bufs=4))
        dram = ctx.enter_context(tc.tile_pool(name="dram", bufs=1,
                                              space="DRAM"))

        # ---------------- persistent SBUF ----------------
        w_qkv_sb = persist.tile([P, DIM // P, QKV_COLS + 1], BF16)
        w_out_sb = persist.tile([P, INNER // P, DIM], BF16)
        qkT = persist.tile([P, 2, B, N], BF16)                    # 16 KB
        v_sb = persist.tile([P, B, KT_PER_B, HLOC, DHEAD + 1], BF16)
        rk8_all = persist.tile([P, B, KT_PER_B, HLOC], F32)
        ones1 = persist.tile([1, DHEAD], BF16)
        warm_a = persist.tile([P, P], BF16)
        warm_b = persist.tile([P, 512], BF16)

        # upfront DMAs (Sync queue) + constants
        nc.sync.dma_start(w_qkv_sb[:], w_qkv.ap())
        nc.sync.dma_start(w_out_sb[:], w_out.ap())
        nc.vector.memset(ones1[:], 1.0)
        nc.vector.memset(warm_a[:], 0.0)
        nc.vector.memset(warm_b[:], 0.0)
        nc.vector.memset(v_sb[:, :, :, :, DHEAD], 1.0)

        # whole-batch x^T stripes (fat DMAs, both issued immediately)
        xt_sb = []
        for bi in range(B):
            t = xt_pool.tile([P, DIM // P, N], BF16, tag="xt")
            nc.sync.dma_start(t[:], x_tr.ap()[:, :, ds(bi * N, N)])
            xt_sb.append(t)

        # x token-row groups (512 tokens each) on the Scalar queue
        xr_view = x_rows.ap().rearrange("(n p) d -> p n d", p=P)

        # PE warm-up: dummy matmuls while DMAs land (~6 us of PE busy)
        warm_ps = warm_ps_pool.tile([P, 512], F32, tag="warm")
        for i in range(14):
            nc.tensor.matmul(warm_ps[:], lhsT=warm_a[:], rhs=warm_b[:],
                             start=True, stop=True)

        # DRAM staging for the 4 AllToAlls (one per batch-half)
        cc_in = [[None] * 2 for _ in range(B)]
        cc_out = [[None] * 2 for _ in range(B)]
        for bi in range(B):
            for h in range(2):
                cc_in[bi][h] = dram.tile([NCORES, P, GRAN], BF16,
                                         name=f"cci{bi}{h}")
                cc_out[bi][h] = dram.tile([NCORES, P, GRAN], BF16,
                                          name=f"cco{bi}{h}")

        oT_b = [None] * B      # per-batch o^T (normalized, bf16)
        st = {}                # per-batch QKV staging tiles
        ssqx_all = persist.tile([P, B, KT_PER_B], F32)

        # ---- x^2 prepass: ACT Square with free accumulation. All 32
        # squares run before the first Exp so the ACT table never thrashes.
        def x2_pass(bi):
            for g in range(4):
                xr = xr_pool.tile([P, 4, DIM], BF16, tag="xr")
                nc.scalar.dma_start(
                    xr[:], xr_view[:, ds(bi * KT_PER_B + 4 * g, 4), :])
                for j in range(4):
                    dump = sq_pool.tile([P, DIM], BF16, tag="sqd")
                    nc.scalar.activation(
                        dump[:], xr[:, j, :], Act.Square,
                        accum_out=ssqx_all[:, bi, 4 * g + j:4 * g + j + 1])

        # ---------------- stage A: QKV for one 128-token tile ------------
        # Per tile: matmuls + raw evacs + squared sums only (no ACT, no
        # reciprocal) -- the norm math is batched per batch to avoid ACT
        # table thrash and per-tile reciprocal overhead.
        def qkv_tile(bi, ti):
            if ti == 0:
                st[bi] = dict(
                    qk_bf=qk_pool.tile([P, KT_PER_B, 2 * P], BF16,
                                       tag="qkbf", name=f"qkbf{bi}"),
                    mu=small.tile([P, KT_PER_B], F32, tag="muall",
                                  name=f"mu{bi}"),
                    ssq=small.tile([P, KT_PER_B, 4], F32, tag="ssqall",
                                   name=f"ssq{bi}"),
                )
            s = st[bi]

            qkv_ps = mm_ps.tile([P, QKV_COLS + 1], F32, tag="mm")
            for o in range(DIM // P):
                nc.tensor.matmul(qkv_ps[:],
                                 lhsT=xt_sb[bi][:, o, ts(ti, P)],
                                 rhs=w_qkv_sb[:, o, :],
                                 start=(o == 0), stop=(o == DIM // P - 1))

            # raw evacs: mu col, q|k bf16, v bf16 (scaled later)
            nc.vector.tensor_copy(s["mu"][:, ti:ti + 1],
                                  qkv_ps[:, QKV_COLS:QKV_COLS + 1])
            nc.vector.tensor_copy(s["qk_bf"][:, ti, :], qkv_ps[:, 0:2 * P])
            nc.vector.tensor_copy(
                v_sb[:, bi, ti, :, 0:DHEAD],
                qkv_ps[:, 2 * P:2 * P + 2 * DHEAD]
                .rearrange("p (h d) -> p h d", d=DHEAD))

            # squared norms per 64-col group
            sq = sq_pool.tile([P, 2 * P], BF16, tag="sq")
            nc.vector.tensor_tensor(sq[:], s["qk_bf"][:, ti, :],
                                    s["qk_bf"][:, ti, :], AluOp.mult)
            nc.vector.reduce_sum(
                s["ssq"][:, ti, :],
                sq[:].rearrange("p (j d) -> p j d", d=DHEAD), axis=AxX)

        # batched per-batch norm math + scale applies + transposes
        def qkv_batch_end(bi):
            s = st[bi]
            # rstd = 1/sqrt(ssqx/1024 - mu^2 + eps)
            musq = small.tile([P, KT_PER_B], F32, tag="musq")
            nc.vector.tensor_tensor(musq[:], s["mu"][:], s["mu"][:],
                                    AluOp.mult)
            varr = small.tile([P, KT_PER_B], F32, tag="varr")
            nc.vector.tensor_scalar(varr[:], ssqx_all[:, bi, :], 1.0 / DIM,
                                    EPS, AluOp.mult, AluOp.add)
            nc.vector.tensor_tensor(varr[:], varr[:], musq[:],
                                    AluOp.subtract)
            rstd = small.tile([P, KT_PER_B], F32, tag="rstd")
            nc.scalar.activation(rstd[:], varr[:], Act.Sqrt)
            nc.vector.reciprocal_approx_fast(rstd[:], rstd[:])
            # rq = 1/max(||q||,1e-12), rk8 = 8/max(||k||,1e-12)
            rq = small.tile([P, KT_PER_B, 2], F32, tag="rq")
            nc.scalar.activation(rq[:], s["ssq"][:, :, 0:2], Act.Sqrt)
            nc.vector.tensor_scalar_max(rq[:], rq[:], 1e-12)
            nc.vector.reciprocal_approx_fast(rq[:], rq[:])
            rk = small.tile([P, KT_PER_B, 2], F32, tag="rk")
            nc.scalar.activation(rk[:], s["ssq"][:, :, 2:4], Act.Sqrt,
                                 scale=1.0 / (SCALE * SCALE))
            nc.vector.tensor_scalar_max(rk[:], rk[:], 1e-12 / SCALE)
            nc.vector.reciprocal_approx_fast(rk8_all[:, bi, :, :], rk[:])
            for ti in range(KT_PER_B):
                nc.vector.tensor_scalar_mul(
                    v_sb[:, bi, ti, :, 0:DHEAD],
                    v_sb[:, bi, ti, :, 0:DHEAD], rstd[:, ti:ti + 1])
                for hh in range(HLOC):
                    nc.vector.tensor_scalar_mul(
                        s["qk_bf"][:, ti, ts(hh, DHEAD)],
                        s["qk_bf"][:, ti, ts(hh, DHEAD)],
                        rq[:, ti, hh:hh + 1])
                nc.sync.dma_start_transpose(qkT[:, :, bi, ts(ti, P)],
                                            s["qk_bf"][:, ti, :])

        # ---------------- stage B: attention for one 512-q block ---------
        def attn_qblock(bi, qb, oU):
            o_ps = []
            for hh in range(HLOC):
                o_ps.append(o_ps_pool.tile([1 + DHEAD, 512], F32,
                                           tag=f"ops{hh}", name=f"ops{hh}"))
            nkt = 4 * (qb + 1)
            for kt in range(nkt):
                d = kt - 4 * qb
                c0 = max(d, 0) * P
                for hh in range(HLOC):
                    hsl = slice(hh * DHEAD, (hh + 1) * DHEAD)
                    st_ps = st_ps_pool.tile([P, 512], F32, tag="stps")
                    nc.tensor.matmul(
                        st_ps[:], lhsT=qkT[hsl, 1, bi, ts(kt, P)],
                        rhs=qkT[hsl, 0, bi, ds(qb * 512, 512)],
                        start=True, stop=True,
                        tile_position=(hh * DHEAD, 0))
                    e_t = e_pool.tile([P, 512], BF16, tag="et")
                    nc.scalar.activation(e_t[:, c0:512], st_ps[:, c0:512],
                                         Act.Exp,
                                         scale=rk8_all[:, bi, kt,
                                                       hh:hh + 1])
                    if d >= 0:
                        nc.gpsimd.affine_select(
                            out=e_t[:, c0:c0 + P], in_=e_t[:, c0:c0 + P],
                            pattern=[[1, P]], compare_op=AluOp.is_ge,
                            fill=0.0, base=0, channel_multiplier=-1)
                    nc.tensor.matmul(
                        o_ps[hh][:, c0:512],
                        lhsT=v_sb[:, bi, kt, hh, :],
                        rhs=e_t[:, c0:512],
                        start=(kt == 0), stop=(kt == nkt - 1))
            for hh in range(HLOC):
                nc.vector.tensor_copy(oU[:, qb % 2, hh, :], o_ps[hh][:])

        # ------- stage C: normalize half-batch, A2A, (out-proj later) ----
        def norm_half(bi, h, oU):
            for q2 in range(2):
                qb = 2 * h + q2
                for hh in range(HLOC):
                    rden = norm_pool.tile([1, 512], BF16, tag="rden")
                    with nc.allow_low_precision(
                            reason="bf16 softmax denom, rel-err budget"):
                        nc.vector.reciprocal(
                            rden[:], oU[DHEAD:DHEAD + 1, q2, hh, :])
                    bc_ps = bc_ps_pool.tile([DHEAD, 512], F32, tag="bc")
                    nc.tensor.matmul(bc_ps[:], lhsT=ones1[:], rhs=rden[:],
                                     start=True, stop=True)
                    nc.vector.tensor_tensor(
                        oT_b[bi][hh * DHEAD:(hh + 1) * DHEAD,
                                 ds(qb * 512, 512)],
                        oU[0:DHEAD, q2, hh, :], bc_ps[:], AluOp.mult)
            nc.sync.dma_start(
                cc_in[bi][h][:].rearrange("s p f -> p s f"),
                oT_b[bi][:, ds(h * 1024, 1024)]
                .rearrange("p (s f) -> p s f", f=GRAN))
            nc.gpsimd.collective_compute(
                "AllToAll", AluOp.bypass,
                replica_groups=[list(range(NCORES))],
                ins=[cc_in[bi][h].opt()], outs=[cc_out[bi][h].opt()])

        def outproj_half(bi, h):
            oA = oA_pool.tile([P, INNER // P, GRAN], BF16, tag="oA")
            nc.sync.dma_start(oA[:],
                              cc_out[bi][h][:].rearrange("s p f -> p s f"))
            yt = out_pool.tile([P, DIM], BF16, tag="yt")
            for half in range(2):
                out_ps = mm_ps.tile([P, 512], F32, tag="mm")
                for o in range(INNER // P):
                    nc.tensor.matmul(
                        out_ps[:], lhsT=oA[:, o, :],
                        rhs=w_out_sb[:, o, ds(half * 512, 512)],
                        start=(o == 0), stop=(o == INNER // P - 1))
                nc.vector.tensor_copy(yt[:, ds(half * 512, 512)], out_ps[:])
            nc.sync.dma_start(y_out.ap()[bi, h], yt[:])

        # ---------------- the schedule ----------------
        x2_pass(0)
        x2_pass(1)
        for ti in range(KT_PER_B):              # QKV batch 0
            qkv_tile(0, ti)
        qkv_batch_end(0)

        oU0 = oU_pool.tile([1 + DHEAD, 2, HLOC, 512], F32, tag="oU")
        oT_b[0] = oT_pool.tile([P, N], BF16, tag="oTb", name="oT0")
        for qb in range(QB_PER_B):              # attn(b0) || QKV(b1)
            if qb == 2:
                oU0b = oU_pool.tile([1 + DHEAD, 2, HLOC, 512], F32,
                                    tag="oU")
            attn_qblock(0, qb, oU0 if qb < 2 else oU0b)
            for ti in range(4 * qb, 4 * qb + 4):
                qkv_tile(1, ti)
            if qb == 1:
                norm_half(0, 0, oU0)
        qkv_batch_end(1)
        norm_half(0, 1, oU0b)

        oU1 = oU_pool.tile([1 + DHEAD, 2, HLOC, 512], F32, tag="oU")
        oT_b[1] = oT_pool.tile([P, N], BF16, tag="oTb", name="oT1")
        attn_qblock(1, 0, oU1)
        attn_qblock(1, 1, oU1)
        outproj_half(0, 0)
        norm_half(1, 0, oU1)
        oU1b = oU_pool.tile([1 + DHEAD, 2, HLOC, 512], F32, tag="oU")
        attn_qblock(1, 2, oU1b)
        outproj_half(0, 1)
        attn_qblock(1, 3, oU1b)
        norm_half(1, 1, oU1b)
        outproj_half(1, 0)
        outproj_half(1, 1)


# ----------------------------------------------------------------------
# Host side
# ----------------------------------------------------------------------

def make_in_maps(x, ln_w, ln_b, W_qkv, W_out):
    """Build the per-core input maps (host-side sharding/marshaling)."""
    import ml_dtypes
    x = np.asarray(x, dtype=np.float32)
    ln_w = np.asarray(ln_w, dtype=np.float32)
    ln_b = np.asarray(ln_b, dtype=np.float32)
    W_qkv = np.asarray(W_qkv, dtype=np.float32)
    W_out = np.asarray(W_out, dtype=np.float32)

    assert np.allclose(ln_b, 0.0), \
        "kernel folds ln_b@W into a bias; nonzero ln_b not wired up"

    x2d = np.ascontiguousarray(x.reshape(NTOK, DIM))
    x_rows = x2d.astype(ml_dtypes.bfloat16)
    # x^T stripes: [128 p, 8 o, 4096 t] with d = 128*o + p
    x_tr = np.ascontiguousarray(
        x2d.T.reshape(DIM // P, P, NTOK).transpose(1, 0, 2)
    ).astype(ml_dtypes.bfloat16)

    w_eff = ln_w[:, None] * W_qkv  # [1024, 3072]
    q_w = w_eff[:, 0 * INNER:1 * INNER]
    k_w = w_eff[:, 1 * INNER:2 * INNER]
    v_w = w_eff[:, 2 * INNER:3 * INNER]
    w_out_r = np.ascontiguousarray(
        W_out.reshape(INNER // P, P, DIM).transpose(1, 0, 2)
    ).astype(ml_dtypes.bfloat16)

    in_maps = []
    for c in range(NCORES):
        h0, h1 = 2 * c, 2 * c + 2
        wq = q_w[:, h0 * DHEAD:h1 * DHEAD]
        wk = k_w[:, h0 * DHEAD:h1 * DHEAD]
        wv = v_w[:, h0 * DHEAD:h1 * DHEAD]
        w_c = np.concatenate([wq, wk, wv], axis=1)      # [1024, 384]
        w_c = w_c - w_c.mean(axis=0, keepdims=True)     # fold LN mean-sub
        mu_col = np.full((DIM, 1), 1.0 / DIM, dtype=np.float32)
        w_c = np.concatenate([w_c, mu_col], axis=1)     # [1024, 385]
        w_c = np.ascontiguousarray(
            w_c.reshape(DIM // P, P, QKV_COLS + 1).transpose(1, 0, 2)
        ).astype(ml_dtypes.bfloat16)
        in_maps.append({
            "x_rows": x_rows,
            "x_tr": x_tr,
            "w_qkv": w_c,
            "w_out": w_out_r,
        })
    return in_maps


def gather_output(results):
    """results: list of per-core {name: array} -> full [2, 2048, 1024]."""
    full = np.empty((B, N, DIM), dtype=np.float32)
    for c in range(NCORES):
        part = np.asarray(results[c]["y_out"], dtype=np.float32)
        for bi in range(B):
            for h in range(2):
                t0 = h * 1024 + c * GRAN
                full[bi, t0:t0 + GRAN, :] = part[bi, h]
    return full


_NC_CACHE = None


def kernel(x, ln_w, ln_b, W_qkv, W_out):
    global _NC_CACHE
    from concourse.bass_utils import run_bass_kernel_spmd
    if _NC_CACHE is None:
        _NC_CACHE = build_kernel()
    in_maps = make_in_maps(x, ln_w, ln_b, W_qkv, W_out)
    res = run_bass_kernel_spmd(_NC_CACHE, in_maps,
                               core_ids=list(range(NCORES)))
    return gather_output(res.results)
